# revision 1
# baseline (speedup 1.0000x reference)
"""AttentiveMatch kernel for Trainium2 (8 NeuronCores, data-parallel over batch).

Reference math (per batch):
    pn = l2norm(p); qn = l2norm(q)
    w  = -(pn @ qn^T) / D          # [S,S]
    mv = (w @ q) / S               # [S,D]
    mn = l2norm(mv)
    out = -mean(pn * mn, -1)       # [S]

Device pipeline (scalars folded, sign flips cancel):
    G^T  = q @ p^T                       [S,S]   matmul 1 (PSUM, fp32)
    A^T  = diag(1/|q_j|) G^T             scale fused into PSUM->SBUF copy
    M^T  = q^T A                         [D,S]   matmul 2 (lhsT = q natural)
    dot_i = p_i . M_i = sum_j (1/|q_j|) (G^T)^2[j,i]   (matmul with rq weights)
    ss_i  = |M_i|^2  = sum_d (M^T)^2[d,i]              (matmul with ones)
    out_i = (1/D) dot_i / (|p_i| sqrt(ss_i))

Each core handles 8 batches; inputs shipped as bf16 in natural and
transposed layouts; all accumulation fp32.
"""

import os
import sys

for _p in ("/opt/trn_rl_repo",):
    if _p not in sys.path:
        sys.path.append(_p)

import numpy as np
import ml_dtypes

import concourse.bacc as bacc
import concourse.mybir as mybir
import concourse.tile as tile
from concourse.bass_utils import run_bass_kernel_spmd

B, S, D = 64, 512, 768
NCORES = 8
BP = B // NCORES          # batches per core
ST = S // 128             # s tiles (4)
KT = D // 128             # d tiles (6)
F32 = mybir.dt.float32
F32R = mybir.dt.float32r
BF16 = mybir.dt.bfloat16
AF = mybir.ActivationFunctionType
ALU = mybir.AluOpType

_NC = None

if os.environ.get("KERNEL_LDW_OPT", "0") == "1":
    import concourse.bass_utils as _bu

    _orig_run_command = _bu.run_command

    def _patched_run_command(cmd, **kw):
        cmd = [
            ("--enable-ldw-opt=true" if c == "--enable-ldw-opt=false" else c)
            for c in cmd
        ]
        return _orig_run_command(cmd, **kw)

    _bu.run_command = _patched_run_command


def _build():
    nc = bacc.Bacc("TRN2", target_bir_lowering=False, debug=False, num_devices=NCORES)
    pn_d = nc.dram_tensor("pn", [BP, 128, ST * D], BF16, kind="ExternalInput")
    qn_d = nc.dram_tensor("qn", [BP, 128, ST * D], BF16, kind="ExternalInput")
    pt_d = nc.dram_tensor("pt", [BP, 128, KT * S], BF16, kind="ExternalInput")
    qt_d = nc.dram_tensor("qt", [BP, 128, KT * S], BF16, kind="ExternalInput")
    out_d = nc.dram_tensor("out", [128, BP * ST], F32, kind="ExternalOutput")

    with tile.TileContext(nc) as tc:
        with (
            tc.tile_pool(name="cst", bufs=1) as cst,
            tc.tile_pool(name="inp", bufs=3) as inp,
            tc.tile_pool(name="ats", bufs=2) as ats,
            tc.tile_pool(name="gps", bufs=3, space="PSUM") as gps,
            tc.tile_pool(name="mps", bufs=3, space="PSUM") as mps,
            tc.tile_pool(name="rps", bufs=1, space="PSUM") as rps,
            tc.tile_pool(name="tps", bufs=1, space="PSUM") as tps,
            tc.tile_pool(name="scr", bufs=2) as scr,
            tc.tile_pool(name="st", bufs=2) as st,
            tc.tile_pool(name="res", bufs=1) as res,
        ):
            wd = res.tile([128, BP * ST], F32)
            ones16 = cst.tile([128, 1], BF16)
            nc.gpsimd.memset(ones16[:], 1.0)
            onef = cst.tile([128, 1], F32)
            nc.gpsimd.memset(onef[:], 1.0)

            for b in range(BP):
                # qt via sync ring, pt via scalar ring (parallel HWDGE rings);
                # batch 0 split into chunks so mm1 starts on the first arrivals
                qt_c = []
                pt_c = []
                nch = 3 if b == 0 else 1
                w = (KT // nch) * S
                for c in range(nch):
                    qc = inp.tile([128, w], BF16, tag=f"qt{c}_{nch}")
                    nc.sync.dma_start(qc[:], qt_d[b, :, c * w:(c + 1) * w])
                    pc = inp.tile([128, w], BF16, tag=f"pt{c}_{nch}")
                    if b == 0:
                        nc.scalar.dma_start(pc[:], pt_d[b, :, c * w:(c + 1) * w])
                    else:
                        nc.sync.dma_start(pc[:], pt_d[b, :, c * w:(c + 1) * w])
                    qt_c.append(qc)
                    pt_c.append(pc)
                q_t = inp.tile([128, ST * D], BF16, tag="q")
                nc.gpsimd.dma_start(q_t[:], qn_d[b])
                p_t = inp.tile([128, ST * D], BF16, tag="p")
                nc.gpsimd.dma_start(p_t[:], pn_d[b])
                kw = 2 if b == 0 else KT

                # q row sum-of-squares via ACT Square+accumulate (needed for rq)
                ssq_q = st.tile([128, ST], F32, tag="ssq_q")
                for t in range(ST):
                    sl = slice(t * D, (t + 1) * D)
                    aq = scr.tile([128, D], BF16, tag="aq")
                    nc.scalar.activation(aq[:], q_t[:, sl], AF.Square,
                                         accum_out=ssq_q[:, t:t + 1])
                sq_q = st.tile([128, ST], F32, tag="sq_q")
                nc.scalar.activation(sq_q[:], ssq_q[:], AF.Sqrt)
                rq = st.tile([128, ST], F32, tag="rq")
                nc.vector.reciprocal(rq[:], sq_q[:])
                sqq16 = st.tile([128, ST], BF16, tag="sqq16")
                nc.vector.tensor_copy(sqq16[:], sq_q[:])

                rows = rps.tile([64, 512], F32, tag="rows")
                trn = tps.tile([128, 2 * ST], F32, tag="trn")

                # mm1: G^T[j,i] = sum_d q[j,d] p[i,d]; A^T = rq * G^T;
                # dot_i = sum_j sq_q[j] (A^T)^2[j,i]  (== sum_j rq_j G^2)
                at_tiles = []
                h_tiles = []
                for j in range(ST):
                    g = gps.tile([128, S], F32, tag="g")
                    for k in range(KT):
                        kc, ko = divmod(k, kw)
                        nc.tensor.matmul(
                            g[:],
                            lhsT=qt_c[kc][:, ko * S + j * 128: ko * S + (j + 1) * 128],
                            rhs=pt_c[kc][:, ko * S: (ko + 1) * S],
                            start=(k == 0), stop=(k == KT - 1),
                        )
                    at = ats.tile([128, S], BF16, tag=f"at{j}")
                    nc.scalar.activation(at[:], g[:], AF.Copy, scale=rq[:, j:j + 1])
                    at_tiles.append(at)
                    h = scr.tile([128, S], BF16, tag=f"h{j}")
                    nc.vector.tensor_mul(h[:], at[:], at[:])
                    h_tiles.append(h)
                for j in range(ST):
                    nc.tensor.matmul(
                        rows[0:1, :], lhsT=sqq16[:, j:j + 1], rhs=h_tiles[j][:],
                        start=(j == 0), stop=(j == ST - 1),
                    )

                # mm2: M^T[d,i] = sum_j q[j,d] A^T[j,i]; ss_row += ones^T @ (M^T)^2
                # ACT squares PSUM directly; DVE sums pairs -> 3 ones-matmuls
                s2_pair = []
                for k in range(KT):
                    mt = mps.tile([128, S], F32, tag="mt")
                    for jt in range(ST):
                        nc.tensor.matmul(
                            mt[:],
                            lhsT=q_t[:, jt * D + k * 128: jt * D + (k + 1) * 128],
                            rhs=at_tiles[jt][:],
                            start=(jt == 0), stop=(jt == ST - 1),
                        )
                    ms = scr.tile([128, S], BF16, tag="ms")
                    nc.vector.tensor_copy(ms[:], mt[:])
                    s2 = scr.tile([128, S], BF16, tag=f"s2{k % 2}")
                    nc.vector.tensor_mul(s2[:], ms[:], ms[:])
                    s2_pair.append(s2)
                    if k % 2 == 1:
                        s2s = scr.tile([128, S], BF16, tag="s2s")
                        nc.vector.tensor_add(s2s[:], s2_pair[0][:], s2_pair[1][:])
                        s2_pair = []
                        nc.tensor.matmul(
                            rows[32:33, :], lhsT=ones16[:], rhs=s2s[:],
                            start=(k == 1), stop=(k == KT - 1),
                        )

                # p row sum-of-squares (only needed for the finals -> late)
                ssq_p = st.tile([128, ST], F32, tag="ssq_p")
                for t in range(ST):
                    sl = slice(t * D, (t + 1) * D)
                    ap_ = scr.tile([128, D], BF16, tag="ap")
                    nc.scalar.activation(ap_[:], p_t[:, sl], AF.Square,
                                         accum_out=ssq_p[:, t:t + 1])
                sq_p = st.tile([128, ST], F32, tag="sq_p")
                nc.scalar.activation(sq_p[:], ssq_p[:], AF.Sqrt)
                rp = st.tile([128, ST], F32, tag="rp")
                nc.vector.reciprocal(rp[:], sq_p[:])

                # transpose the two [1,512] rows into [128, ST] columns
                rowsb = st.tile([64, 512], F32, tag="rowsb")
                nc.vector.tensor_copy(rowsb[:], rows[:])
                for c in range(ST):
                    nc.tensor.matmul(
                        trn[:, c:c + 1],
                        lhsT=rowsb[0:1, c * 128:(c + 1) * 128],
                        rhs=onef[0:1, :], start=(c == 0), stop=False,
                    )
                for c in range(ST):
                    nc.tensor.matmul(
                        trn[:, ST + c: ST + c + 1],
                        lhsT=rowsb[32:33, c * 128:(c + 1) * 128],
                        rhs=onef[32:33, :], start=(c == 0), stop=(c == ST - 1),
                    )

                # wd = (1/D) * dot / (sq_p * sqrt(ss));  sqrt(D^2 ss) folds 1/D
                sd = st.tile([128, ST], F32, tag="sd")
                nc.scalar.activation(sd[:], trn[:, ST: 2 * ST], AF.Sqrt,
                                     scale=float(D) * float(D))
                rs = st.tile([128, ST], F32, tag="rs")
                nc.vector.reciprocal(rs[:], sd[:])
                w1 = st.tile([128, ST], F32, tag="w1")
                nc.vector.tensor_mul(w1[:], trn[:, 0:ST], rp[:])
                nc.vector.tensor_mul(wd[:, b * ST: (b + 1) * ST], w1[:], rs[:])

            nc.sync.dma_start(out_d[:], wd[:])
    nc.compile()
    return nc


def _get_nc():
    global _NC
    if _NC is None:
        _NC = _build()
    return _NC


def _prep_inputs(p, q):
    p = np.asarray(p, dtype=np.float32)
    q = np.asarray(q, dtype=np.float32)
    p16 = p.astype(ml_dtypes.bfloat16)
    q16 = q.astype(ml_dtypes.bfloat16)

    # natural: [core, b, part, t*D + d] with s = t*128 + part
    def nat(x):
        return np.ascontiguousarray(
            x.reshape(NCORES, BP, ST, 128, D).transpose(0, 1, 3, 2, 4)
        ).reshape(NCORES, BP, 128, ST * D)

    # transposed: [core, b, part, k*S + i] with d = k*128 + part
    def tr(x):
        return np.ascontiguousarray(
            x.reshape(NCORES, BP, S, KT, 128).transpose(0, 1, 4, 3, 2)
        ).reshape(NCORES, BP, 128, KT * S)

    pn, qn, pt, qt = nat(p16), nat(q16), tr(p16), tr(q16)
    return [
        {"pn": pn[c], "qn": qn[c], "pt": pt[c], "qt": qt[c]}
        for c in range(NCORES)
    ]


def _postprocess(results):
    o = np.stack([np.asarray(r["out"], dtype=np.float32) for r in results])
    # o[c, part, b*ST + t] is out for batch c*BP+b at i = t*128 + part
    o = o.reshape(NCORES, 128, BP, ST).transpose(0, 2, 3, 1).reshape(B, 1, S)
    return np.ascontiguousarray(o)


def _run(inputs, trace=False, **kw):
    nc = _get_nc()
    in_maps = _prep_inputs(inputs["p"], inputs["q"])
    res = run_bass_kernel_spmd(nc, in_maps, list(range(NCORES)), trace=trace, **kw)
    return _postprocess(res.results), res


def kernel(p, q):
    out, _ = _run({"p": p, "q": q})
    return out



# revision 31
# speedup vs baseline: 1.2283x; 1.2283x over previous
"""AttentiveMatch kernel for Trainium2 (8 NeuronCores, data-parallel over batch).

Reference math (per batch):
    pn = l2norm(p); qn = l2norm(q)
    w  = -(pn @ qn^T) / D          # [S,S]
    mv = (w @ q) / S               # [S,D]
    mn = l2norm(mv)
    out = -mean(pn * mn, -1)       # [S]

Signs/scalars fold away: out_i = (1/D) * (p_i . M_i) / (|p_i| |M_i|)
with M_i = sum_j (G_ji / |q_j|) q_j and G = q p^T.

fp8 pipeline with the row-norm folded into q on the host:
    qs = fp8(sqrt(1/|q8_j|) * q)   shipped in natural + transposed layouts
    b  = qs @ p8^T                 [S,S]  mm1, fp8 DoubleRow -> = sqrt(rq)*G
    b8 = fp8(b)                    PSUM->SBUF copy
    M^T = qs^T b8                  [D,S]  mm2, fp8 DoubleRow
    dot_i = sum_j b8[j,i]^2        ones-weight fp8 DoubleRow matmul row
    ss_i  = sum_d (M^T)^2[d,i]     bf16 Square + ones matmul row
    out_i = dot_i / (D |p8_i| sqrt(ss_i))

Rows are PE-transposed into [128, ST] columns; finals run columnar.
"""

import sys

for _p in ("/opt/trn_rl_repo",):
    if _p not in sys.path:
        sys.path.append(_p)

import numpy as np
import ml_dtypes

import concourse.bacc as bacc
import concourse.mybir as mybir
import concourse.tile as tile
from concourse.bass_utils import run_bass_kernel_spmd

B, S, D = 64, 512, 768
NCORES = 8
BP = B // NCORES          # batches per core
ST = S // 128             # s tiles (4)
KT = D // 128             # d tiles (6)
F32 = mybir.dt.float32
BF16 = mybir.dt.bfloat16
F8 = mybir.dt.float8e4
AF = mybir.ActivationFunctionType
ALU = mybir.AluOpType
DR = mybir.MatmulPerfMode.DoubleRow
NPF8 = ml_dtypes.float8_e4m3

_NC = None


def _build():
    nc = bacc.Bacc("TRN2", target_bir_lowering=False, debug=False, num_devices=NCORES)
    # weight layouts keep each [128, 2, 128] DoubleRow pair contiguous
    qst_d = nc.dram_tensor("qst", [BP, 128, KT // 2, ST, 2, 128], F8,
                           kind="ExternalInput")
    pt_d = nc.dram_tensor("pt", [BP, 128, KT, S], F8, kind="ExternalInput")
    qs_d = nc.dram_tensor("qs", [BP, 128, ST // 2, KT, 2, 128], F8,
                          kind="ExternalInput")
    rp_d = nc.dram_tensor("rp", [128, BP, ST], F32, kind="ExternalInput")
    out_d = nc.dram_tensor("out", [128, BP * ST], F32, kind="ExternalOutput")

    with tile.TileContext(nc) as tc:
        with (
            tc.tile_pool(name="cst", bufs=1) as cst,
            tc.tile_pool(name="inp", bufs=3) as inp,
            tc.tile_pool(name="bsb", bufs=2) as bsb,
            tc.tile_pool(name="s2b", bufs=2) as s2b,
            tc.tile_pool(name="st", bufs=2) as st,
            tc.tile_pool(name="gps", bufs=2, space="PSUM") as gps,
            tc.tile_pool(name="mps", bufs=3, space="PSUM") as mps,
            tc.tile_pool(name="rps", bufs=2, space="PSUM") as rps,
            tc.tile_pool(name="tps", bufs=1, space="PSUM") as tps,
            tc.tile_pool(name="res", bufs=1) as res,
        ):
            ones16 = cst.tile([128, 1], BF16)
            nc.gpsimd.memset(ones16[:], 1.0)
            onef = cst.tile([128, 1], F32)
            nc.gpsimd.memset(onef[:], 1.0)
            rptile = res.tile([128, BP, ST], F32)
            nc.sync.dma_start(rptile[:], rp_d[:])
            wd = res.tile([128, BP * ST], F32)

            # per-batch state carried across the software pipeline
            st_rows = [None] * BP
            st_s2 = [None] * BP
            st_h = [None] * BP

            def load(b, split):
                nch = 3 if split else 1
                w = KT // 2 // nch   # k-pairs per chunk
                qc, pc = [], []
                for c in range(nch):
                    t = inp.tile([128, w, ST, 2, 128], F8, tag=f"qst{c}_{nch}")
                    nc.sync.dma_start(t[:], qst_d[b, :, c * w:(c + 1) * w])
                    qc.append(t)
                    t = inp.tile([128, 2 * w, S], F8, tag=f"pt{c}_{nch}")
                    nc.scalar.dma_start(
                        t[:], pt_d[b, :, 2 * c * w:2 * (c + 1) * w, :])
                    pc.append(t)
                qn = inp.tile([128, ST // 2, KT, 2, 128], F8, tag="qs")
                nc.gpsimd.dma_start(qn[:], qs_d[b])
                return qc, pc, qn

            def ss_rows(b):
                # 6 ones-matmuls: rows[32] = sum_d s2 (bf16)
                rows = st_rows[b]
                s2 = st_s2[b]
                for k in range(KT):
                    nc.tensor.matmul(
                        rows[32:33, :], lhsT=ones16[:], rhs=s2[:, k, :],
                        start=(k == 0), stop=(k == KT - 1),
                    )

            def finish_pe(b):
                # transpose the two [1,512] rows into [128, ST] columns
                rows = st_rows[b]
                rowsb = st.tile([33, S], F32, tag="rowsb")
                nc.vector.tensor_copy(rowsb[0:1, :], rows[0:1, :])
                nc.vector.tensor_copy(rowsb[32:33, :], rows[32:33, :])
                trn = tps.tile([128, 2 * ST], F32, tag="trn")
                for c in range(ST):
                    nc.tensor.matmul(
                        trn[:, c:c + 1],
                        lhsT=rowsb[0:1, c * 128:(c + 1) * 128],
                        rhs=onef[0:1, :], start=(c == 0), stop=False,
                    )
                for c in range(ST):
                    nc.tensor.matmul(
                        trn[:, ST + c:ST + c + 1],
                        lhsT=rowsb[32:33, c * 128:(c + 1) * 128],
                        rhs=onef[32:33, :], start=False, stop=(c == ST - 1),
                    )
                return trn

            def finish_vec(b, trn):
                # columnar finals into wd
                sd = st.tile([128, ST], F32, tag="sd")
                nc.scalar.activation(sd[:], trn[:, ST:2 * ST], AF.Sqrt,
                                     scale=float(D) * float(D))
                rs = st.tile([128, ST], F32, tag="rs")
                nc.vector.reciprocal(rs[:], sd[:])
                w1 = st.tile([128, ST], F32, tag="w1")
                nc.vector.tensor_mul(w1[:], trn[:, 0:ST], rptile[:, b, :])
                nc.vector.tensor_mul(wd[:, b * ST:(b + 1) * ST], w1[:], rs[:])

            loads = load(0, True)
            for b in range(BP):
                qc, pc, qn = loads
                kw = (KT // 2) // len(qc)  # k-pairs per chunk

                # mm1: b_pre[j,i] = sum_d qs[j,d] p8[i,d], DoubleRow k-pairs
                bp = bsb.tile([128, ST, S], F8, tag="bp")
                h = s2b.tile([128, ST, S], BF16, tag="h")
                st_h[b] = h
                for jt in range(ST):
                    g = gps.tile([128, S], F32, tag="g")
                    for c in range(KT // 2):
                        kc, ko = divmod(c, kw)
                        nc.tensor.matmul(
                            g[:],
                            lhsT=qc[kc][:, ko, jt],
                            rhs=pc[kc][:, 2 * ko:2 * ko + 2, :],
                            start=(c == 0), stop=(c == KT // 2 - 1),
                            perf_mode=DR,
                        )
                    nc.vector.tensor_copy(bp[:, jt, :], g[:])
                    nc.scalar.activation(h[:, jt, :], g[:], AF.Square)

                # software pipeline: prev batch's ss reduction on the PE
                # here, after its s2 tiles have certainly landed
                if b > 0:
                    ss_rows(b - 1)

                # prefetch next batch while mm2 runs
                if b + 1 < BP:
                    loads = load(b + 1, False)

                rows = rps.tile([64, S], F32, tag="rows")
                st_rows[b] = rows

                # mm2: mt[k] = sum_j qs[j,d] b8[j,i] (fp8 DoubleRow jt-pairs)
                s2 = s2b.tile([128, KT, S], BF16, tag="s2")
                st_s2[b] = s2
                for k in range(KT):
                    mt = mps.tile([128, S], F32, tag="mt")
                    for jp in range(ST // 2):
                        nc.tensor.matmul(
                            mt[:],
                            lhsT=qn[:, jp, k],
                            rhs=bp[:, 2 * jp:2 * jp + 2, :],
                            start=(jp == 0), stop=(jp == ST // 2 - 1),
                            perf_mode=DR,
                        )
                    # M^T squares for the ss reduction (single PSUM read)
                    nc.scalar.activation(s2[:, k, :], mt[:], AF.Square)

                # dot rows: rows[0] = sum_j h (bf16 ones-matmuls)
                for jt in range(ST):
                    nc.tensor.matmul(
                        rows[0:1, :], lhsT=ones16[:], rhs=h[:, jt, :],
                        start=(jt == 0), stop=(jt == ST - 1),
                    )

                if b > 0:
                    trn = finish_pe(b - 1)
                    finish_vec(b - 1, trn)

            ss_rows(BP - 1)
            trn = finish_pe(BP - 1)
            finish_vec(BP - 1, trn)
            nc.sync.dma_start(out_d[:], wd[:])
    nc.compile()
    return nc


def _get_nc():
    global _NC
    if _NC is None:
        _NC = _build()
    return _NC


def _prep_inputs(p, q):
    p = np.asarray(p, dtype=np.float32)
    q = np.asarray(q, dtype=np.float32)
    p8 = p.astype(NPF8)
    p8f = p8.astype(np.float32)
    q8f = q.astype(NPF8).astype(np.float32)
    rq = 1.0 / np.sqrt((q8f * q8f).sum(-1))            # [B,S]
    rp = (1.0 / np.sqrt((p8f * p8f).sum(-1))).astype(np.float32)
    qs8 = (np.sqrt(rq)[..., None] * q).astype(NPF8)    # [B,S,D] fp8

    # mm1 weights: [core, b, dpart, kp, jt, e, jc] with d = (2kp+e)*128+dpart,
    # j = jt*128 + jc  (each [128, 2, 128] DoubleRow pair contiguous)
    qst = np.ascontiguousarray(
        qs8.reshape(NCORES, BP, ST, 128, KT // 2, 2, 128)
        .transpose(0, 1, 6, 4, 2, 5, 3)
    )
    # mm1 moving: [core, b, part, k, i] with d = k*128 + part
    pt = np.ascontiguousarray(
        p8.reshape(NCORES, BP, S, KT, 128).transpose(0, 1, 4, 3, 2)
    )
    # mm2 weights: [core, b, jpart, jp, k, e, dc] with j = (2jp+e)*128+jpart,
    # d = k*128 + dc
    qsn = np.ascontiguousarray(
        qs8.reshape(NCORES, BP, ST // 2, 2, 128, KT, 128)
        .transpose(0, 1, 4, 2, 5, 3, 6)
    )
    # rp columns: [core, part, b, t] with i = t*128 + part
    rpc = np.ascontiguousarray(
        rp.reshape(NCORES, BP, ST, 128).transpose(0, 3, 1, 2)
    )
    return [
        {"qst": qst[c], "pt": pt[c], "qs": qsn[c], "rp": rpc[c]}
        for c in range(NCORES)
    ]


def _postprocess(results):
    o = np.stack([np.asarray(r["out"], dtype=np.float32) for r in results])
    # o[c, part, b*ST + t] is out for batch c*BP+b at i = t*128 + part
    o = o.reshape(NCORES, 128, BP, ST).transpose(0, 2, 3, 1).reshape(B, 1, S)
    return np.ascontiguousarray(o)


def _run(inputs, trace=False, **kw):
    nc = _get_nc()
    in_maps = _prep_inputs(inputs["p"], inputs["q"])
    res = run_bass_kernel_spmd(nc, in_maps, list(range(NCORES)), trace=trace, **kw)
    return _postprocess(res.results), res


def kernel(p, q):
    out, _ = _run({"p": p, "q": q})
    return out


# revision 36
# speedup vs baseline: 1.3036x; 1.0613x over previous
"""AttentiveMatch kernel for Trainium2 (8 NeuronCores, data-parallel over batch).

Reference math (per batch):
    pn = l2norm(p); qn = l2norm(q)
    w  = -(pn @ qn^T) / D          # [S,S]
    mv = (w @ q) / S               # [S,D]
    mn = l2norm(mv)
    out = -mean(pn * mn, -1)       # [S]

Signs/scalars fold away: out_i = (1/D) * (p_i . M_i) / (|p_i| |M_i|)
with M_i = sum_j (G_ji / |q_j|) q_j and G = q p^T.

fp8 pipeline with the row-norm folded into q on the host:
    qs = fp8(sqrt(1/|q8_j|) * q)   shipped in natural + transposed layouts
    b  = qs @ p8^T                 [S,S]  mm1, fp8 DoubleRow -> = sqrt(rq)*G
    b8 = fp8(b)                    PSUM->SBUF copy
    M^T = qs^T b8                  [D,S]  mm2, fp8 DoubleRow
    dot_i = sum_j b8[j,i]^2        ones-weight fp8 DoubleRow matmul row
    ss_i  = sum_d (M^T)^2[d,i]     bf16 Square + ones matmul row
    out_i = dot_i / (D |p8_i| sqrt(ss_i))

Rows are PE-transposed into [128, ST] columns; finals run columnar.
"""

import os
import sys

for _p in ("/opt/trn_rl_repo",):
    if _p not in sys.path:
        sys.path.append(_p)

import numpy as np
import ml_dtypes

import concourse.bacc as bacc
import concourse.mybir as mybir
import concourse.tile as tile
from concourse.bass_utils import run_bass_kernel_spmd

B, S, D = 64, 512, 768
NCORES = 8
BP = B // NCORES          # batches per core
ST = S // 128             # s tiles (4)
KT = D // 128             # d tiles (6)
F32 = mybir.dt.float32
BF16 = mybir.dt.bfloat16
F8 = mybir.dt.float8e4
AF = mybir.ActivationFunctionType
ALU = mybir.AluOpType
DR = mybir.MatmulPerfMode.DoubleRow
NPF8 = ml_dtypes.float8_e4m3

_NC = None

if os.environ.get("KERNEL_LDW_OPT", "0") == "1":
    # pipeline LdWeights with the previous matmul's stream
    import concourse.bass_utils as _bu

    _orig_run_command = _bu.run_command

    def _patched_run_command(cmd, **kw):
        cmd = [
            ("--enable-ldw-opt=true" if c == "--enable-ldw-opt=false" else c)
            for c in cmd
        ]
        return _orig_run_command(cmd, **kw)

    _bu.run_command = _patched_run_command


def _build():
    nc = bacc.Bacc("TRN2", target_bir_lowering=False, debug=False, num_devices=NCORES)
    # weight layouts keep each [128, 2, 128] DoubleRow pair contiguous
    qst_d = nc.dram_tensor("qst", [BP, 128, KT // 2, ST, 2, 128], F8,
                           kind="ExternalInput")
    pt_d = nc.dram_tensor("pt", [BP, 128, KT, S], F8, kind="ExternalInput")
    qs_d = nc.dram_tensor("qs", [BP, 128, ST // 2, KT, 2, 128], F8,
                          kind="ExternalInput")
    rp_d = nc.dram_tensor("rp", [128, BP, ST], F32, kind="ExternalInput")
    out_d = nc.dram_tensor("out", [128, BP * ST], F32, kind="ExternalOutput")

    with tile.TileContext(nc) as tc:
        with (
            tc.tile_pool(name="cst", bufs=1) as cst,
            tc.tile_pool(name="inp", bufs=3) as inp,
            tc.tile_pool(name="bsb", bufs=2) as bsb,
            tc.tile_pool(name="s2b", bufs=2) as s2b,
            tc.tile_pool(name="st", bufs=2) as st,
            tc.tile_pool(name="gps", bufs=2, space="PSUM") as gps,
            tc.tile_pool(name="mps", bufs=3, space="PSUM") as mps,
            tc.tile_pool(name="rps", bufs=2, space="PSUM") as rps,
            tc.tile_pool(name="tps", bufs=1, space="PSUM") as tps,
            tc.tile_pool(name="res", bufs=1) as res,
        ):
            ones16 = cst.tile([128, 1], BF16)
            nc.gpsimd.memset(ones16[:], 1.0)
            onef = cst.tile([128, 1], F32)
            nc.gpsimd.memset(onef[:], 1.0)
            rptile = res.tile([128, BP, ST], F32)
            nc.sync.dma_start(rptile[:], rp_d[:])
            wd = res.tile([128, BP * ST], F32)

            # per-batch state carried across the software pipeline
            st_rows = [None] * BP
            st_s2 = [None] * BP
            st_h = [None] * BP

            def load(b, split):
                nch = 3 if split else 1
                w = KT // 2 // nch   # k-pairs per chunk
                qc, pc = [], []
                for c in range(nch):
                    t = inp.tile([128, w, ST, 2, 128], F8, tag=f"qst{c}_{nch}")
                    nc.sync.dma_start(t[:], qst_d[b, :, c * w:(c + 1) * w])
                    qc.append(t)
                    t = inp.tile([128, 2 * w, S], F8, tag=f"pt{c}_{nch}")
                    nc.scalar.dma_start(
                        t[:], pt_d[b, :, 2 * c * w:2 * (c + 1) * w, :])
                    pc.append(t)
                qn = inp.tile([128, ST // 2, KT, 2, 128], F8, tag="qs")
                nc.gpsimd.dma_start(qn[:], qs_d[b])
                return qc, pc, qn

            def ss_rows(b):
                # 6 ones-matmuls: rows[32] = sum_d s2 (bf16)
                rows = st_rows[b]
                s2 = st_s2[b]
                for k in range(KT):
                    nc.tensor.matmul(
                        rows[32:33, :], lhsT=ones16[:], rhs=s2[:, k, :],
                        start=(k == 0), stop=(k == KT - 1),
                    )

            def finish_pe(b):
                # transpose the two [1,512] rows into [128, ST] columns
                rows = st_rows[b]
                rowsb = st.tile([33, S], F32, tag="rowsb")
                nc.vector.tensor_copy(rowsb[0:1, :], rows[0:1, :])
                nc.vector.tensor_copy(rowsb[32:33, :], rows[32:33, :])
                trn = tps.tile([128, 2 * ST], F32, tag="trn")
                for c in range(ST):
                    nc.tensor.matmul(
                        trn[:, c:c + 1],
                        lhsT=rowsb[0:1, c * 128:(c + 1) * 128],
                        rhs=onef[0:1, :], start=(c == 0), stop=False,
                    )
                for c in range(ST):
                    nc.tensor.matmul(
                        trn[:, ST + c:ST + c + 1],
                        lhsT=rowsb[32:33, c * 128:(c + 1) * 128],
                        rhs=onef[32:33, :], start=False, stop=(c == ST - 1),
                    )
                return trn

            def finish_vec(b, trn):
                # columnar finals into wd
                sd = st.tile([128, ST], F32, tag="sd")
                nc.scalar.activation(sd[:], trn[:, ST:2 * ST], AF.Sqrt,
                                     scale=float(D) * float(D))
                rs = st.tile([128, ST], F32, tag="rs")
                nc.vector.reciprocal(rs[:], sd[:])
                w1 = st.tile([128, ST], F32, tag="w1")
                nc.vector.tensor_mul(w1[:], trn[:, 0:ST], rptile[:, b, :])
                nc.vector.tensor_mul(wd[:, b * ST:(b + 1) * ST], w1[:], rs[:])

            loads = load(0, True)
            for b in range(BP):
                qc, pc, qn = loads
                kw = (KT // 2) // len(qc)  # k-pairs per chunk

                # mm1: b_pre[j,i] = sum_d qs[j,d] p8[i,d], DoubleRow k-pairs
                bp = bsb.tile([128, ST, S], F8, tag="bp")
                h = s2b.tile([128, ST, S], BF16, tag="h")
                st_h[b] = h
                for jt in range(ST):
                    g = gps.tile([128, S], F32, tag="g")
                    for c in range(KT // 2):
                        kc, ko = divmod(c, kw)
                        nc.tensor.matmul(
                            g[:],
                            lhsT=qc[kc][:, ko, jt],
                            rhs=pc[kc][:, 2 * ko:2 * ko + 2, :],
                            start=(c == 0), stop=(c == KT // 2 - 1),
                            perf_mode=DR,
                        )
                    nc.vector.tensor_copy(bp[:, jt, :], g[:])
                    # h = b^2 in bf16, split across ACT (from PSUM) and DVE
                    if jt < 2:
                        nc.scalar.activation(h[:, jt, :], g[:], AF.Square)
                    else:
                        nc.vector.scalar_tensor_tensor(
                            h[:, jt, :], bp[:, jt, :], 1.0, bp[:, jt, :],
                            ALU.mult, ALU.mult)

                # software pipeline: prev batch's ss reduction on the PE
                # here, after its s2 tiles have certainly landed
                if b > 0:
                    ss_rows(b - 1)

                # prefetch next batch while mm2 runs
                if b + 1 < BP:
                    loads = load(b + 1, False)

                rows = rps.tile([64, S], F32, tag="rows")
                st_rows[b] = rows

                # mm2: mt[k] = sum_j qs[j,d] b8[j,i] (fp8 DoubleRow jt-pairs)
                s2 = s2b.tile([128, KT, S], BF16, tag="s2")
                st_s2[b] = s2
                for k in range(KT):
                    mt = mps.tile([128, S], F32, tag="mt")
                    for jp in range(ST // 2):
                        nc.tensor.matmul(
                            mt[:],
                            lhsT=qn[:, jp, k],
                            rhs=bp[:, 2 * jp:2 * jp + 2, :],
                            start=(jp == 0), stop=(jp == ST // 2 - 1),
                            perf_mode=DR,
                        )
                    # M^T squares for the ss reduction (single PSUM read)
                    nc.scalar.activation(s2[:, k, :], mt[:], AF.Square)

                # dot rows: rows[0] = sum_j h (bf16 ones-matmuls)
                for jt in range(ST):
                    nc.tensor.matmul(
                        rows[0:1, :], lhsT=ones16[:], rhs=h[:, jt, :],
                        start=(jt == 0), stop=(jt == ST - 1),
                    )

                if b > 0:
                    trn = finish_pe(b - 1)
                    finish_vec(b - 1, trn)

            ss_rows(BP - 1)
            trn = finish_pe(BP - 1)
            finish_vec(BP - 1, trn)
            nc.sync.dma_start(out_d[:], wd[:])
    nc.compile()
    return nc


def _get_nc():
    global _NC
    if _NC is None:
        _NC = _build()
    return _NC


def _prep_inputs(p, q):
    p = np.asarray(p, dtype=np.float32)
    q = np.asarray(q, dtype=np.float32)
    p8 = p.astype(NPF8)
    p8f = p8.astype(np.float32)
    q8f = q.astype(NPF8).astype(np.float32)
    rq = 1.0 / np.sqrt((q8f * q8f).sum(-1))            # [B,S]
    rp = (1.0 / np.sqrt((p8f * p8f).sum(-1))).astype(np.float32)
    qs8 = (np.sqrt(rq)[..., None] * q).astype(NPF8)    # [B,S,D] fp8

    # mm1 weights: [core, b, dpart, kp, jt, e, jc] with d = (2kp+e)*128+dpart,
    # j = jt*128 + jc  (each [128, 2, 128] DoubleRow pair contiguous)
    qst = np.ascontiguousarray(
        qs8.reshape(NCORES, BP, ST, 128, KT // 2, 2, 128)
        .transpose(0, 1, 6, 4, 2, 5, 3)
    )
    # mm1 moving: [core, b, part, k, i] with d = k*128 + part
    pt = np.ascontiguousarray(
        p8.reshape(NCORES, BP, S, KT, 128).transpose(0, 1, 4, 3, 2)
    )
    # mm2 weights: [core, b, jpart, jp, k, e, dc] with j = (2jp+e)*128+jpart,
    # d = k*128 + dc
    qsn = np.ascontiguousarray(
        qs8.reshape(NCORES, BP, ST // 2, 2, 128, KT, 128)
        .transpose(0, 1, 4, 2, 5, 3, 6)
    )
    # rp columns: [core, part, b, t] with i = t*128 + part
    rpc = np.ascontiguousarray(
        rp.reshape(NCORES, BP, ST, 128).transpose(0, 3, 1, 2)
    )
    return [
        {"qst": qst[c], "pt": pt[c], "qs": qsn[c], "rp": rpc[c]}
        for c in range(NCORES)
    ]


def _postprocess(results):
    o = np.stack([np.asarray(r["out"], dtype=np.float32) for r in results])
    # o[c, part, b*ST + t] is out for batch c*BP+b at i = t*128 + part
    o = o.reshape(NCORES, 128, BP, ST).transpose(0, 2, 3, 1).reshape(B, 1, S)
    return np.ascontiguousarray(o)


def _run(inputs, trace=False, **kw):
    nc = _get_nc()
    in_maps = _prep_inputs(inputs["p"], inputs["q"])
    res = run_bass_kernel_spmd(nc, in_maps, list(range(NCORES)), trace=trace, **kw)
    return _postprocess(res.results), res


def kernel(p, q):
    out, _ = _run({"p": p, "q": q})
    return out


# revision 37
# speedup vs baseline: 1.3809x; 1.0593x over previous
"""AttentiveMatch kernel for Trainium2 (8 NeuronCores, data-parallel over batch).

Reference math (per batch):
    pn = l2norm(p); qn = l2norm(q)
    w  = -(pn @ qn^T) / D          # [S,S]
    mv = (w @ q) / S               # [S,D]
    mn = l2norm(mv)
    out = -mean(pn * mn, -1)       # [S]

Signs/scalars fold away: out_i = (1/D) * (p_i . M_i) / (|p_i| |M_i|)
with M_i = sum_j (G_ji / |q_j|) q_j and G = q p^T.

fp8 pipeline with the row-norm folded into q on the host:
    qs = fp8(sqrt(1/|q8_j|) * q)   shipped in natural + transposed layouts
    b  = qs @ p8^T                 [S,S]  mm1, fp8 DoubleRow -> = sqrt(rq)*G
    b8 = fp8(b)                    PSUM->SBUF copy
    M^T = qs^T b8                  [D,S]  mm2, fp8 DoubleRow
    dot_i = sum_j b8[j,i]^2        ones-weight fp8 DoubleRow matmul row
    ss_i  = sum_d (M^T)^2[d,i]     bf16 Square + ones matmul row
    out_i = dot_i / (D |p8_i| sqrt(ss_i))

Rows are PE-transposed into [128, ST] columns; finals run columnar.
"""

import os
import sys

for _p in ("/opt/trn_rl_repo",):
    if _p not in sys.path:
        sys.path.append(_p)

import numpy as np
import ml_dtypes

import concourse.bacc as bacc
import concourse.mybir as mybir
import concourse.tile as tile
from concourse.bass_utils import run_bass_kernel_spmd

B, S, D = 64, 512, 768
NCORES = 8
BP = B // NCORES          # batches per core
ST = S // 128             # s tiles (4)
KT = D // 128             # d tiles (6)
F32 = mybir.dt.float32
BF16 = mybir.dt.bfloat16
F8 = mybir.dt.float8e4
AF = mybir.ActivationFunctionType
ALU = mybir.AluOpType
DR = mybir.MatmulPerfMode.DoubleRow
NPF8 = ml_dtypes.float8_e4m3

_NC = None

if os.environ.get("KERNEL_LDW_OPT", "0") == "1":
    # pipeline LdWeights with the previous matmul's stream
    import concourse.bass_utils as _bu

    _orig_run_command = _bu.run_command

    def _patched_run_command(cmd, **kw):
        cmd = [
            ("--enable-ldw-opt=true" if c == "--enable-ldw-opt=false" else c)
            for c in cmd
        ]
        return _orig_run_command(cmd, **kw)

    _bu.run_command = _patched_run_command


def _build():
    nc = bacc.Bacc("TRN2", target_bir_lowering=False, debug=False, num_devices=NCORES)
    # weight layouts keep each [128, 2, 128] DoubleRow pair contiguous
    qst_d = nc.dram_tensor("qst", [BP, 128, KT // 2, ST, 2, 128], F8,
                           kind="ExternalInput")
    pt_d = nc.dram_tensor("pt", [BP, 128, KT, S], F8, kind="ExternalInput")
    qs_d = nc.dram_tensor("qs", [BP, 128, ST // 2, KT, 2, 128], F8,
                          kind="ExternalInput")
    rp_d = nc.dram_tensor("rp", [BP, S], F32, kind="ExternalInput")
    out_d = nc.dram_tensor("out", [BP, S], F32, kind="ExternalOutput")

    with tile.TileContext(nc) as tc:
        with (
            tc.tile_pool(name="cst", bufs=1) as cst,
            tc.tile_pool(name="inp", bufs=3) as inp,
            tc.tile_pool(name="bsb", bufs=2) as bsb,
            tc.tile_pool(name="s2b", bufs=2) as s2b,
            tc.tile_pool(name="st", bufs=2) as st,
            tc.tile_pool(name="gps", bufs=2, space="PSUM") as gps,
            tc.tile_pool(name="mps", bufs=4, space="PSUM") as mps,
            tc.tile_pool(name="rps", bufs=2, space="PSUM") as rps,
            tc.tile_pool(name="res", bufs=1) as res,
        ):
            ones16 = cst.tile([128, 1], BF16)
            nc.gpsimd.memset(ones16[:], 1.0)

            # per-batch state carried across the software pipeline
            st_rows = [None] * BP
            st_s2 = [None] * BP
            st_h = [None] * BP
            st_rpt = [None] * BP

            def load(b, split):
                nch = 3 if split else 1
                w = KT // 2 // nch   # k-pairs per chunk
                qc, pc = [], []
                for c in range(nch):
                    t = inp.tile([128, w, ST, 2, 128], F8, tag=f"qst{c}_{nch}")
                    nc.sync.dma_start(t[:], qst_d[b, :, c * w:(c + 1) * w])
                    qc.append(t)
                    t = inp.tile([128, 2 * w, S], F8, tag=f"pt{c}_{nch}")
                    nc.scalar.dma_start(
                        t[:], pt_d[b, :, 2 * c * w:2 * (c + 1) * w, :])
                    pc.append(t)
                qn = inp.tile([128, ST // 2, KT, 2, 128], F8, tag="qs")
                nc.gpsimd.dma_start(qn[:], qs_d[b])
                rpt = st.tile([1, S], F32, tag="rpt")
                nc.sync.dma_start(rpt[:], rp_d[b:b + 1, :])
                st_rpt[b] = rpt
                return qc, pc, qn

            def ss_rows(b):
                # 6 ones-matmuls: rows[32] = sum_d s2 (bf16)
                rows = st_rows[b]
                s2 = st_s2[b]
                for k in range(KT):
                    nc.tensor.matmul(
                        rows[32:33, :], lhsT=ones16[:], rhs=s2[:, k, :],
                        start=(k == 0), stop=(k == KT - 1),
                    )

            def finish(b):
                # row-wise finals straight off the PSUM rows:
                # out = dot * rp / (D * sqrt(ss))
                rows = st_rows[b]
                sd = st.tile([1, S], F32, tag="sd")
                nc.scalar.activation(sd[:], rows[32:33, :], AF.Sqrt,
                                     scale=float(D) * float(D))
                rs = st.tile([1, S], F32, tag="rs")
                nc.vector.reciprocal(rs[:], sd[:])
                w1 = st.tile([1, S], F32, tag="w1")
                nc.vector.tensor_mul(w1[:], rows[0:1, :], st_rpt[b][:])
                ow = st.tile([1, S], F32, tag="ow")
                nc.gpsimd.tensor_mul(ow[:], w1[:], rs[:])
                nc.gpsimd.dma_start(out_d[b:b + 1, :], ow[:])

            loads = load(0, True)
            for b in range(BP):
                qc, pc, qn = loads
                kw = (KT // 2) // len(qc)  # k-pairs per chunk

                # mm1: b_pre[j,i] = sum_d qs[j,d] p8[i,d], DoubleRow k-pairs
                bp = bsb.tile([128, ST, S], F8, tag="bp")
                h = s2b.tile([128, ST, S], BF16, tag="h")
                st_h[b] = h
                for jt in range(ST):
                    g = gps.tile([128, S], F32, tag="g")
                    for c in range(KT // 2):
                        kc, ko = divmod(c, kw)
                        nc.tensor.matmul(
                            g[:],
                            lhsT=qc[kc][:, ko, jt],
                            rhs=pc[kc][:, 2 * ko:2 * ko + 2, :],
                            start=(c == 0), stop=(c == KT // 2 - 1),
                            perf_mode=DR,
                        )
                    nc.vector.tensor_copy(bp[:, jt, :], g[:])
                    # h = b^2 in bf16, split across ACT (from PSUM) and DVE
                    if jt < 2:
                        nc.scalar.activation(h[:, jt, :], g[:], AF.Square)
                    else:
                        nc.vector.scalar_tensor_tensor(
                            h[:, jt, :], bp[:, jt, :], 1.0, bp[:, jt, :],
                            ALU.mult, ALU.mult)

                # software pipeline: prev batch's ss reduction on the PE
                # here, after its s2 tiles have certainly landed
                if b > 0:
                    ss_rows(b - 1)

                # prefetch next batch while mm2 runs
                if b + 1 < BP:
                    loads = load(b + 1, False)

                rows = rps.tile([64, S], F32, tag="rows")
                st_rows[b] = rows

                # mm2: mt[k] = sum_j qs[j,d] b8[j,i] (fp8 DoubleRow jt-pairs)
                s2 = s2b.tile([128, KT, S], BF16, tag="s2")
                st_s2[b] = s2
                for k in range(KT):
                    mt = mps.tile([128, S], F32, tag="mt")
                    for jp in range(ST // 2):
                        nc.tensor.matmul(
                            mt[:],
                            lhsT=qn[:, jp, k],
                            rhs=bp[:, 2 * jp:2 * jp + 2, :],
                            start=(jp == 0), stop=(jp == ST // 2 - 1),
                            perf_mode=DR,
                        )
                    # M^T squares for the ss reduction (single PSUM read)
                    nc.scalar.activation(s2[:, k, :], mt[:], AF.Square)

                # dot rows: rows[0] = sum_j h (bf16 ones-matmuls)
                for jt in range(ST):
                    nc.tensor.matmul(
                        rows[0:1, :], lhsT=ones16[:], rhs=h[:, jt, :],
                        start=(jt == 0), stop=(jt == ST - 1),
                    )

                if b > 0:
                    finish(b - 1)

            ss_rows(BP - 1)
            finish(BP - 1)
    nc.compile()
    return nc


def _get_nc():
    global _NC
    if _NC is None:
        _NC = _build()
    return _NC


def _prep_inputs(p, q):
    p = np.asarray(p, dtype=np.float32)
    q = np.asarray(q, dtype=np.float32)
    p8 = p.astype(NPF8)
    p8f = p8.astype(np.float32)
    q8f = q.astype(NPF8).astype(np.float32)
    rq = 1.0 / np.sqrt((q8f * q8f).sum(-1))            # [B,S]
    rp = (1.0 / np.sqrt((p8f * p8f).sum(-1))).astype(np.float32)
    qs8 = (np.sqrt(rq)[..., None] * q).astype(NPF8)    # [B,S,D] fp8

    # mm1 weights: [core, b, dpart, kp, jt, e, jc] with d = (2kp+e)*128+dpart,
    # j = jt*128 + jc  (each [128, 2, 128] DoubleRow pair contiguous)
    qst = np.ascontiguousarray(
        qs8.reshape(NCORES, BP, ST, 128, KT // 2, 2, 128)
        .transpose(0, 1, 6, 4, 2, 5, 3)
    )
    # mm1 moving: [core, b, part, k, i] with d = k*128 + part
    pt = np.ascontiguousarray(
        p8.reshape(NCORES, BP, S, KT, 128).transpose(0, 1, 4, 3, 2)
    )
    # mm2 weights: [core, b, jpart, jp, k, e, dc] with j = (2jp+e)*128+jpart,
    # d = k*128 + dc
    qsn = np.ascontiguousarray(
        qs8.reshape(NCORES, BP, ST // 2, 2, 128, KT, 128)
        .transpose(0, 1, 4, 2, 5, 3, 6)
    )
    rpc = np.ascontiguousarray(rp.reshape(NCORES, BP, S))
    return [
        {"qst": qst[c], "pt": pt[c], "qs": qsn[c], "rp": rpc[c]}
        for c in range(NCORES)
    ]


def _postprocess(results):
    o = np.stack([np.asarray(r["out"], dtype=np.float32) for r in results])
    return np.ascontiguousarray(o.reshape(B, 1, S))


def _run(inputs, trace=False, **kw):
    nc = _get_nc()
    in_maps = _prep_inputs(inputs["p"], inputs["q"])
    res = run_bass_kernel_spmd(nc, in_maps, list(range(NCORES)), trace=trace, **kw)
    return _postprocess(res.results), res


def kernel(p, q):
    out, _ = _run({"p": p, "q": q})
    return out


# revision 38
# speedup vs baseline: 1.5794x; 1.1438x over previous
"""AttentiveMatch kernel for Trainium2 (8 NeuronCores, data-parallel over batch).

Reference math (per batch):
    pn = l2norm(p); qn = l2norm(q)
    w  = -(pn @ qn^T) / D          # [S,S]
    mv = (w @ q) / S               # [S,D]
    mn = l2norm(mv)
    out = -mean(pn * mn, -1)       # [S]

Signs/scalars fold away: out_i = (1/D) * (p_i . M_i) / (|p_i| |M_i|)
with M_i = sum_j (G_ji / |q_j|) q_j and G = q p^T.

fp8 pipeline with the row-norm folded into q on the host:
    qs = fp8(sqrt(1/|q8_j|) * q)   shipped in natural + transposed layouts
    b  = qs @ p8^T                 [S,S]  mm1, fp8 DoubleRow -> = sqrt(rq)*G
    b8 = fp8(b)                    PSUM->SBUF copy
    M^T = qs^T b8                  [D,S]  mm2, fp8 DoubleRow
    dot_i = sum_j b8[j,i]^2        ones-weight fp8 DoubleRow matmul row
    ss_i  = sum_d (M^T)^2[d,i]     bf16 Square + ones matmul row
    out_i = dot_i / (D |p8_i| sqrt(ss_i))

Rows are PE-transposed into [128, ST] columns; finals run columnar.
"""

import os
import sys

for _p in ("/opt/trn_rl_repo",):
    if _p not in sys.path:
        sys.path.append(_p)

import numpy as np
import ml_dtypes

import concourse.bacc as bacc
import concourse.mybir as mybir
import concourse.tile as tile
from concourse.bass_utils import run_bass_kernel_spmd

B, S, D = 64, 512, 768
NCORES = 8
BP = B // NCORES          # batches per core
ST = S // 128             # s tiles (4)
KT = D // 128             # d tiles (6)
F32 = mybir.dt.float32
BF16 = mybir.dt.bfloat16
F8 = mybir.dt.float8e4
AF = mybir.ActivationFunctionType
ALU = mybir.AluOpType
DR = mybir.MatmulPerfMode.DoubleRow
NPF8 = ml_dtypes.float8_e4m3

_NC = None

if os.environ.get("KERNEL_LDW_OPT", "0") == "1":
    # pipeline LdWeights with the previous matmul's stream
    import concourse.bass_utils as _bu

    _orig_run_command = _bu.run_command

    def _patched_run_command(cmd, **kw):
        cmd = [
            ("--enable-ldw-opt=true" if c == "--enable-ldw-opt=false" else c)
            for c in cmd
        ]
        return _orig_run_command(cmd, **kw)

    _bu.run_command = _patched_run_command


def _build():
    nc = bacc.Bacc("TRN2", target_bir_lowering=False, debug=False, num_devices=NCORES)
    # weight layouts keep each [128, 2, 128] DoubleRow pair contiguous
    qst_d = nc.dram_tensor("qst", [BP, 128, KT // 2, ST, 2, 128], F8,
                           kind="ExternalInput")
    pt_d = nc.dram_tensor("pt", [BP, 128, KT, S], F8, kind="ExternalInput")
    qs_d = nc.dram_tensor("qs", [BP, 128, ST // 2, KT, 2, 128], F8,
                          kind="ExternalInput")
    rp_d = nc.dram_tensor("rp", [BP, S], F32, kind="ExternalInput")
    out_d = nc.dram_tensor("out", [BP, S], F32, kind="ExternalOutput")

    with tile.TileContext(nc) as tc:
        with (
            tc.tile_pool(name="cst", bufs=1) as cst,
            tc.tile_pool(name="inp", bufs=3) as inp,
            tc.tile_pool(name="bsb", bufs=2) as bsb,
            tc.tile_pool(name="s2b", bufs=2) as s2b,
            tc.tile_pool(name="st", bufs=2) as st,
            tc.tile_pool(name="gps", bufs=2, space="PSUM") as gps,
            tc.tile_pool(name="mps", bufs=4, space="PSUM") as mps,
            tc.tile_pool(name="rps", bufs=2, space="PSUM") as rps,
            tc.tile_pool(name="res", bufs=1) as res,
        ):
            ones16 = cst.tile([128, 1], BF16)
            nc.gpsimd.memset(ones16[:], 1.0)

            # per-batch state carried across the software pipeline
            st_rows = [None] * BP
            st_s2 = [None] * BP
            st_h = [None] * BP
            st_rpt = [None] * BP

            def load(b, split):
                nch = 3 if split else 1
                w = KT // 2 // nch   # k-pairs per chunk
                qc, pc = [], []
                for c in range(nch):
                    t = inp.tile([128, w, ST, 2, 128], F8, tag=f"qst{c}_{nch}")
                    nc.sync.dma_start(t[:], qst_d[b, :, c * w:(c + 1) * w])
                    qc.append(t)
                    t = inp.tile([128, 2 * w, S], F8, tag=f"pt{c}_{nch}")
                    nc.scalar.dma_start(
                        t[:], pt_d[b, :, 2 * c * w:2 * (c + 1) * w, :])
                    pc.append(t)
                qn = inp.tile([128, ST // 2, KT, 2, 128], F8, tag="qs")
                nc.gpsimd.dma_start(qn[:], qs_d[b])
                rpt = st.tile([1, S], F32, tag="rpt")
                nc.sync.dma_start(rpt[:], rp_d[b:b + 1, :])
                st_rpt[b] = rpt
                return qc, pc, qn

            def ss_rows(b):
                # 6 ones-matmuls: rows[32] = sum_d s2 (bf16)
                rows = st_rows[b]
                s2 = st_s2[b]
                for k in range(KT):
                    nc.tensor.matmul(
                        rows[32:33, :], lhsT=ones16[:], rhs=s2[:, k, :],
                        start=(k == 0), stop=(k == KT - 1),
                    )

            def finish(b):
                # row-wise finals straight off the PSUM rows:
                # out = dot * rp / (D * sqrt(ss))
                rows = st_rows[b]
                sd = st.tile([1, S], F32, tag="sd")
                nc.scalar.activation(sd[:], rows[32:33, :], AF.Sqrt,
                                     scale=float(D) * float(D))
                rs = st.tile([1, S], F32, tag="rs")
                nc.vector.reciprocal_approx_fast(rs[:], sd[:])
                w1 = st.tile([1, S], F32, tag="w1")
                nc.vector.tensor_mul(w1[:], rows[0:1, :], st_rpt[b][:])
                ow = st.tile([1, S], F32, tag="ow")
                nc.gpsimd.tensor_mul(ow[:], w1[:], rs[:])
                nc.gpsimd.dma_start(out_d[b:b + 1, :], ow[:])

            loads = load(0, True)
            for b in range(BP):
                qc, pc, qn = loads
                kw = (KT // 2) // len(qc)  # k-pairs per chunk

                # mm1: b_pre[j,i] = sum_d qs[j,d] p8[i,d], DoubleRow k-pairs
                bp = bsb.tile([128, ST, S], F8, tag="bp")
                h = s2b.tile([128, ST, S], BF16, tag="h")
                st_h[b] = h
                for jt in range(ST):
                    g = gps.tile([128, S], F32, tag="g")
                    for c in range(KT // 2):
                        kc, ko = divmod(c, kw)
                        nc.tensor.matmul(
                            g[:],
                            lhsT=qc[kc][:, ko, jt],
                            rhs=pc[kc][:, 2 * ko:2 * ko + 2, :],
                            start=(c == 0), stop=(c == KT // 2 - 1),
                            perf_mode=DR,
                        )
                    nc.vector.tensor_copy(bp[:, jt, :], g[:])
                    # h = b^2 in bf16, split across ACT (from PSUM) and DVE
                    if jt < 2:
                        nc.scalar.activation(h[:, jt, :], g[:], AF.Square)
                    else:
                        nc.vector.scalar_tensor_tensor(
                            h[:, jt, :], bp[:, jt, :], 1.0, bp[:, jt, :],
                            ALU.mult, ALU.mult)

                # software pipeline: prev batch's ss reduction on the PE
                # here, after its s2 tiles have certainly landed
                if b > 0:
                    ss_rows(b - 1)

                # prefetch next batch while mm2 runs
                if b + 1 < BP:
                    loads = load(b + 1, False)

                rows = rps.tile([64, S], F32, tag="rows")
                st_rows[b] = rows

                # mm2: mt[k] = sum_j qs[j,d] b8[j,i] (fp8 DoubleRow jt-pairs)
                s2 = s2b.tile([128, KT, S], BF16, tag="s2")
                st_s2[b] = s2
                for k in range(KT):
                    mt = mps.tile([128, S], F32, tag="mt")
                    for jp in range(ST // 2):
                        nc.tensor.matmul(
                            mt[:],
                            lhsT=qn[:, jp, k],
                            rhs=bp[:, 2 * jp:2 * jp + 2, :],
                            start=(jp == 0), stop=(jp == ST // 2 - 1),
                            perf_mode=DR,
                        )
                    # M^T squares for the ss reduction (single PSUM read)
                    nc.scalar.activation(s2[:, k, :], mt[:], AF.Square)

                # dot rows: rows[0] = sum_j h (bf16 ones-matmuls)
                for jt in range(ST):
                    nc.tensor.matmul(
                        rows[0:1, :], lhsT=ones16[:], rhs=h[:, jt, :],
                        start=(jt == 0), stop=(jt == ST - 1),
                    )

                if b > 0:
                    finish(b - 1)

            ss_rows(BP - 1)
            finish(BP - 1)
    nc.compile()
    return nc


def _get_nc():
    global _NC
    if _NC is None:
        _NC = _build()
    return _NC


def _prep_inputs(p, q):
    p = np.asarray(p, dtype=np.float32)
    q = np.asarray(q, dtype=np.float32)
    p8 = p.astype(NPF8)
    p8f = p8.astype(np.float32)
    q8f = q.astype(NPF8).astype(np.float32)
    rq = 1.0 / np.sqrt((q8f * q8f).sum(-1))            # [B,S]
    rp = (1.0 / np.sqrt((p8f * p8f).sum(-1))).astype(np.float32)
    qs8 = (np.sqrt(rq)[..., None] * q).astype(NPF8)    # [B,S,D] fp8

    # mm1 weights: [core, b, dpart, kp, jt, e, jc] with d = (2kp+e)*128+dpart,
    # j = jt*128 + jc  (each [128, 2, 128] DoubleRow pair contiguous)
    qst = np.ascontiguousarray(
        qs8.reshape(NCORES, BP, ST, 128, KT // 2, 2, 128)
        .transpose(0, 1, 6, 4, 2, 5, 3)
    )
    # mm1 moving: [core, b, part, k, i] with d = k*128 + part
    pt = np.ascontiguousarray(
        p8.reshape(NCORES, BP, S, KT, 128).transpose(0, 1, 4, 3, 2)
    )
    # mm2 weights: [core, b, jpart, jp, k, e, dc] with j = (2jp+e)*128+jpart,
    # d = k*128 + dc
    qsn = np.ascontiguousarray(
        qs8.reshape(NCORES, BP, ST // 2, 2, 128, KT, 128)
        .transpose(0, 1, 4, 2, 5, 3, 6)
    )
    rpc = np.ascontiguousarray(rp.reshape(NCORES, BP, S))
    return [
        {"qst": qst[c], "pt": pt[c], "qs": qsn[c], "rp": rpc[c]}
        for c in range(NCORES)
    ]


def _postprocess(results):
    o = np.stack([np.asarray(r["out"], dtype=np.float32) for r in results])
    return np.ascontiguousarray(o.reshape(B, 1, S))


def _run(inputs, trace=False, **kw):
    nc = _get_nc()
    in_maps = _prep_inputs(inputs["p"], inputs["q"])
    res = run_bass_kernel_spmd(nc, in_maps, list(range(NCORES)), trace=trace, **kw)
    return _postprocess(res.results), res


def kernel(p, q):
    out, _ = _run({"p": p, "q": q})
    return out
